# revision 20
# baseline (speedup 1.0000x reference)
"""Hyperbolic (Poincare-ball) average pooling 1D — Trainium2 Bass kernel.

Problem: x (16, 256, 16384) f32, kernel=stride=4, manifold dim = channels (256).
Math (c=1), per window position:
    n2   = sum_C x^2                     (per input position)
    r    = 1/(1-n2)                      (gamma*xK = 2*r*x ; gamma = 2r-1)
    num  = sum_j r_j x_j  (window of 4)  ; den = sum_j r_j ; D = den - 2
    out  = num * g,  g = 1/(D + sqrt(D^2 - s)),  s = sum_C num^2

bf16 end-to-end (tolerance 2e-2; lands ~3e-3), halving DMA both ways
(roofline ~59us/core).  n2 via DVE square + tree adds (L1 DVE 2x, L2/L3
GpSimd, short reduce DVE).  Window-sum on PE: each q-slot's 128 positions
fold into a DISJOINT 32-row output band, so the weight matrix is just a
[128, 32] band (r at column p//4) matmul'd into a partition-offset PSUM
slice — one broadcast-multiply builds all 8 bands per tile.  PSUM is never
evacuated: s via ScalarE Square+accum from PSUM, output scaled straight
from PSUM by ScalarE activation Copy with per-partition scale AP.
Software pipelining: matmuls of r-group k run while n2 of group k+1
streams; each pair's post-math (s/D/g/out) is deferred one pair.

Sharding: data-parallel over batch (2 rows/core, 8 cores). Host
pre-transposes each core's slice to (positions, channels+ones+pad) bf16.
"""

import sys

sys.path.insert(0, "/opt/trn_rl_repo")

import copy
import numpy as np
import ml_dtypes

import bass_rust
import concourse.bass as bass
import concourse.mybir as mybir
from concourse import tile
from concourse.bass_utils import run_bass_kernel_spmd
from contextlib import ExitStack

F32 = mybir.dt.float32
BF16 = mybir.dt.bfloat16
NP_BF16 = ml_dtypes.bfloat16

B, C, L = 16, 256, 16384
KERN = 4
T = L // KERN            # 4096 out positions per batch row
N_CORES = 8
B_PER = B // N_CORES     # 2
POS = B_PER * L          # 32768 input positions per core
OPOS = POS // KERN       # 8192 out positions per core
CPC = C + 2              # 258: channels + ones column + zero pad
Q = 8                    # q-slots per x-tile
TILE_POS = 128 * Q       # 1024 input positions per x-tile
N_TILES = POS // TILE_POS  # 32
RG = 2                   # x-tiles per r-group (= one matmul pair)

AF = mybir.ActivationFunctionType
ALU = mybir.AluOpType


def _split_multi_waits(nc, max_waits=1):
    """walrus in this container rejects >1 sync-wait on one instruction
    (setupSyncWait 'Too many sync wait commands'); split extras into
    preceding single-wait NOPs on the same engine."""
    n_new = 0
    for bb in nc.m.functions[0].blocks:
        new_list = []
        for inst in bb.instructions:
            si = getattr(inst, "sync_info", None)
            if si is not None and si.on_wait and len(si.on_wait) > max_waits:
                extra = si.on_wait[:-max_waits]
                si_keep = si.on_wait[-max_waits:]
                for w in extra:
                    nop = mybir.InstNoOp(
                        name=f"{inst.name}-wsplit{n_new}", ins=[], outs=[]
                    )
                    nop.engine = inst.engine
                    nsi = copy.deepcopy(si)
                    nsi.on_wait = [w]
                    nsi.on_update = []
                    nop.sync_info = nsi
                    new_list.append(nop)
                    n_new += 1
                si.on_wait = si_keep
            new_list.append(inst)
        bb.instructions = new_list
    return n_new


def _register_const_ap(nc, value):
    t = nc.alloc_sbuf_tensor(f"const-float32-{value}", [128, 1], F32)
    nc.gpsimd.memset(t.ap(), value)
    nc.const_aps.aps[(F32, value)] = t.ap()


def build_nc(split_waits=True):
    nc = bass.Bass()
    _register_const_ap(nc, 2.0)
    _register_const_ap(nc, -2.0)
    nc.all_engine_barrier()
    xt = nc.declare_dram_parameter("xt", [N_TILES, 128, Q * CPC], BF16, isOutput=False)
    mb = nc.declare_dram_parameter("mb", [128, 32], BF16, isOutput=False)
    # p-major output so paired tiles DMA with matching iteration order
    out = nc.declare_dram_parameter("out", [128, N_TILES, 2 * 256], BF16, isOutput=True)

    with tile.TileContext(nc) as tc:
        with ExitStack() as ctx:
            xpool = ctx.enter_context(tc.tile_pool(name="x", bufs=10))
            sqpool = ctx.enter_context(tc.tile_pool(name="sq", bufs=3))
            t1pool = ctx.enter_context(tc.tile_pool(name="t1", bufs=3))
            t2pool = ctx.enter_context(tc.tile_pool(name="t2", bufs=6))
            bdpool = ctx.enter_context(tc.tile_pool(name="bd", bufs=3))
            stpool = ctx.enter_context(tc.tile_pool(name="st", bufs=3))
            scpool = ctx.enter_context(tc.tile_pool(name="sc", bufs=4))
            opool = ctx.enter_context(tc.tile_pool(name="o", bufs=3))
            mkpool = ctx.enter_context(tc.tile_pool(name="mk", bufs=1))
            pspool = ctx.enter_context(tc.tile_pool(name="ps", bufs=8, space="PSUM"))

            mb_t = mkpool.tile([128, 32], BF16, tag="mb")
            nc.sync.dma_start(mb_t[:], mb[:, :])
            mb_bc = (
                mb_t[:]
                .rearrange("p (a b t) -> p a b t", a=1, b=1)
                .broadcast_to([128, 4, 2, 32])
            )

            # [128, Q, 64] weight tiles: slot q's 32-wide band sits at column
            # 32*(q%2); zeros elsewhere are written once and persist across
            # the pool's buffer rotation.
            def band_ap(wt_full):
                # band offset for q = 2a+b is 64q + 32b = 128a + 96b
                return bass_rust.AP(
                    tensor=wt_full.tensor,
                    offset=wt_full.offset,
                    ap=[list(wt_full.ap[0]), [128, 4], [96, 2], [1, 32]],
                )

            for _ in range(3):
                wt = bdpool.tile([128, Q, 64], BF16, tag="bd")
                nc.vector.memset(wt[:], 0.0)

            n_rgroups = N_TILES // RG

            def stage1(rg):
                """load + square + tree-reduce -> n2 -> r for RG tiles.
                Op-major emission so each engine's consumers trail their
                producers by a full sub-stage; a few squares go to ScalarE
                to level DVE/Scalar busy."""
                n2_g = stpool.tile([128, RG * Q], F32, tag="n2")
                xts, sqs, t1s, t2s, t3s = [], [], [], [], []
                for j in range(RG):
                    i = rg * RG + j
                    x_t = xpool.tile([128, Q, CPC], BF16, tag="x")
                    nc.sync.dma_start(
                        x_t[:], xt[i].rearrange("p (q c) -> p q c", q=Q)
                    )
                    xts.append(x_t)
                for j in range(RG):
                    sq_t = sqpool.tile([128, Q, 256], BF16, tag="sq")
                    if rg % 5 in (1, 3) and j == 0:
                        nc.scalar.activation(
                            sq_t[:], xts[j][:, :, 0:256], AF.Square
                        )
                    else:
                        nc.vector.tensor_tensor(
                            out=sq_t[:],
                            in0=xts[j][:, :, 0:256],
                            in1=xts[j][:, :, 0:256],
                            op=ALU.mult,
                        )
                    sqs.append(sq_t)
                for j in range(RG):
                    t1_t = t1pool.tile([128, Q, 128], BF16, tag="t1")
                    nc.vector.tensor_tensor(
                        out=t1_t[:],
                        in0=sqs[j][:, :, 0:128],
                        in1=sqs[j][:, :, 128:256],
                        op=ALU.add,
                    )
                    t1s.append(t1_t)
                for j in range(RG):
                    t2_t = t2pool.tile([128, Q, 64], BF16, tag="t2")
                    nc.vector.tensor_tensor(
                        out=t2_t[:],
                        in0=t1s[j][:, :, 0:64],
                        in1=t1s[j][:, :, 64:128],
                        op=ALU.add,
                    )
                    t2s.append(t2_t)
                for j in range(RG):
                    t3_t = t2pool.tile([128, Q, 32], BF16, tag="t3")
                    nc.gpsimd.tensor_tensor(
                        out=t3_t[:],
                        in0=t2s[j][:, :, 0:32],
                        in1=t2s[j][:, :, 32:64],
                        op=ALU.add,
                    )
                    t3s.append(t3_t)
                for j in range(RG):
                    nc.vector.tensor_reduce(
                        n2_g[:, j * Q : (j + 1) * Q],
                        t3s[j][:],
                        axis=mybir.AxisListType.X,
                        op=ALU.add,
                    )
                om_g = stpool.tile([128, RG * Q], F32, tag="om")
                nc.vector.tensor_scalar(
                    out=om_g[:],
                    in0=n2_g[:],
                    scalar1=-1.0,
                    scalar2=1.0,
                    op0=ALU.mult,
                    op1=ALU.add,
                )
                r_g = stpool.tile([128, RG * Q], F32, tag="r")
                nc.vector.reciprocal(r_g[:], om_g[:])
                return xts, r_g

            def emit_mm(rg, xts, r_g):
                """band build + 8 banded matmuls for a 2-tile group."""
                ps_list = []
                for jj in range(2):
                    j = jj
                    x_t = xts[j]
                    band = bdpool.tile([128, Q, 64], BF16, tag="bd")
                    r_bc = (
                        r_g[:, j * Q : (j + 1) * Q]
                        .rearrange("p (a b o) -> p a b o", a=4, o=1)
                        .broadcast_to([128, 4, 2, 32])
                    )
                    nc.gpsimd.tensor_tensor(
                        out=band_ap(band[:].rearrange("p q t -> p (q t)")),
                        in0=mb_bc,
                        in1=r_bc,
                        op=ALU.mult,
                    )
                    for bk in range(2):
                        ps = pspool.tile([128, CPC], F32, tag="ps")
                        for ql in range(4):
                            q = 4 * bk + ql
                            nc.tensor.matmul(
                                ps[64 * (ql // 2) : 64 * (ql // 2) + 64, :],
                                band[:, q, :],
                                x_t[:, q, :],
                                start=(ql % 2 == 0),
                                stop=(ql % 2 == 1),
                            )
                        ps_list.append(ps)
                return ps_list

            def emit_sD(ps_list):
                """s = sum num^2 and D = den-2 straight off PSUM (issued
                right after the matmuls so the deferred g-chain never
                waits on ScalarE's queue)."""
                d_s = scpool.tile([128, 4], F32, tag="d")
                s_s = scpool.tile([128, 4], F32, tag="s")
                for col, ps in enumerate(ps_list):
                    sq_scr = scpool.tile([128, 256], BF16, tag="sqs")
                    nc.scalar.activation(
                        sq_scr[:],
                        ps[:, 0:256],
                        AF.Square,
                        accum_out=s_s[:, col : col + 1],
                    )
                    nc.scalar.activation(
                        d_s[:, col : col + 1],
                        ps[:, 256:257],
                        AF.Identity,
                        bias=-2.0,
                    )
                d2 = scpool.tile([128, 4], F32, tag="d2")
                nc.gpsimd.tensor_tensor(out=d2[:], in0=d_s[:], in1=d_s[:], op=ALU.mult)
                return d_s, s_s, d2

            def emit_post(rg, ps_list, d_s, s_s, d2):
                """g-chain, output scale + DMA for a 2-tile group."""
                # g = 1/(D + sqrt(D^2 - s)) for 4 halves at once
                qq = scpool.tile([128, 4], F32, tag="qq")
                nc.gpsimd.tensor_tensor(out=qq[:], in0=d2[:], in1=s_s[:], op=ALU.subtract)
                u = scpool.tile([128, 4], F32, tag="u")
                nc.scalar.activation(u[:], qq[:], AF.Sqrt)
                du = scpool.tile([128, 4], F32, tag="du")
                nc.gpsimd.tensor_tensor(out=du[:], in0=d_s[:], in1=u[:], op=ALU.add)
                g_s = scpool.tile([128, 4], F32, tag="g")
                nc.vector.reciprocal(g_s[:], du[:])

                i0 = rg * RG
                o_t = opool.tile([128, 2, 512], BF16, tag="o")
                for jj in range(2):
                    for bk in range(2):
                        col = 2 * jj + bk
                        nc.scalar.activation(
                            o_t[:, jj, bk * 256 : (bk + 1) * 256],
                            ps_list[col][:, 0:256],
                            AF.Copy,
                            scale=g_s[:, col : col + 1],
                        )
                nc.sync.dma_start(out[:, i0 : i0 + 2, :], o_t[:])

            prev = None      # (rg, xts, r_g) awaiting matmuls
            pending = None   # (rg, ps_list, d_s, s_s) awaiting g-chain/out
            for rg in range(n_rgroups):
                cur = stage1(rg)
                if prev is not None:
                    prg, xts, r_g = prev
                    ps_list = emit_mm(prg, xts, r_g)
                    d_s, s_s, d2 = emit_sD(ps_list)
                    if pending is not None:
                        emit_post(*pending)
                    pending = (prg, ps_list, d_s, s_s, d2)
                prev = (rg, *cur)
            prg, xts, r_g = prev
            ps_list = emit_mm(prg, xts, r_g)
            d_s, s_s, d2 = emit_sD(ps_list)
            if pending is not None:
                emit_post(*pending)
            emit_post(prg, ps_list, d_s, s_s, d2)

    if split_waits:
        _split_multi_waits(nc)
    return nc


_NC_CACHE = None


def _get_nc():
    global _NC_CACHE
    if _NC_CACHE is None:
        _NC_CACHE = build_nc()
    return _NC_CACHE


def _make_mask():
    m = np.zeros((128, 32), dtype=NP_BF16)
    m[np.arange(128), np.arange(128) // 4] = 1.0
    return m


def prepare_core_inputs(x):
    """x: (16, 256, 16384) f32 -> list of per-core input dicts."""
    mask = _make_mask()
    in_maps = []
    for k in range(N_CORES):
        xs = x[k * B_PER : (k + 1) * B_PER]  # (2, 256, L)
        xt = np.empty((POS, CPC), dtype=NP_BF16)
        xt[:, :C] = xs.transpose(0, 2, 1).reshape(POS, C).astype(NP_BF16)
        xt[:, C] = 1.0
        xt[:, C + 1] = 0.0
        # partition-major per-tile layout: (tile, p, q*CPC)
        xt = np.ascontiguousarray(
            xt.reshape(N_TILES, Q, 128, CPC).transpose(0, 2, 1, 3)
        ).reshape(N_TILES, 128, Q * CPC)
        in_maps.append({"xt": xt, "mb": mask})
    return in_maps


def assemble_output(results):
    outs = []
    for k in range(N_CORES):
        o = results[k]["out"]  # (128, N_TILES, 2*256) bf16, p-major
        o = np.asarray(o).astype(np.float32)
        # o[m, i, h*256+c] -> out position i*256 + h*128 + m, channel c
        o = o.reshape(128, N_TILES, 2, 256).transpose(1, 2, 0, 3).reshape(OPOS, 256)
        outs.append(o.reshape(B_PER, T, C).transpose(0, 2, 1))
    return np.ascontiguousarray(np.concatenate(outs, axis=0))


def kernel(x):
    x = np.ascontiguousarray(x, dtype=np.float32)
    nc = _get_nc()
    in_maps = prepare_core_inputs(x)
    res = run_bass_kernel_spmd(nc, in_maps, core_ids=list(range(N_CORES)))
    return assemble_output(res.results)
